# revision 1
# baseline (speedup 1.0000x reference)
"""Two-layer GraphSAGE (mean aggregation) on 8 Trainium2 NeuronCores.

Strategy (matches the dst-partitioning hint):
- Nodes are partitioned by destination across 8 cores (12500 nodes each,
  padded to 12544 = 98*128). Each core owns the edges whose dst lands in
  its slice, pre-sorted/bucketed by (core, dst-tile) on the host.
- x is replicated to every core in a padded layout so src indices are
  identical for both layers. Layer-1 aggregation gathers x[src] rows with
  large indirect DMAs, reduces them per 128-dst tile with indicator
  matmuls on the TensorEngine (indicator built on DVE from host-provided
  dst offsets), and applies mean + the two dense 128x128 matmuls.
- Between layers, each core's h slice is AllGathered so layer 2 can
  gather h[src] for remote sources. Layer-2 self term reads the local
  (pre-AllGather) slice.

kernel(**inputs) -> np.ndarray takes the FULL inputs and returns the FULL
[100000, 128] output; all sharding/unsharding happens inside.
"""

import math
import os

import numpy as np

P = 128
NCORES = 8


def _prep_edges(edge_index: np.ndarray, n_nodes: int, npc: int, tpc: int):
    """Bucket edges by (owner core, dst tile); pad each tile to whole
    128-edge chunks (uniform chunk count across cores per tile so the SPMD
    program is identical on every core).

    Returns (ch, coloff, ncols, esrc, edst):
      ch[t]    : number of 128-edge chunks for dst tile t (max over cores)
      coloff[t]: starting column of tile t in the packed arrays
      esrc     : [8, 128, ncols] int32, padded-global src ids (pad = 0)
      edst     : [8, 128, ncols] float32, dst offset within tile (pad = -1)
    """
    npc_pad = tpc * P
    src = edge_index[0].astype(np.int64)
    dst = edge_index[1].astype(np.int64)
    srcpad = ((src // npc) * npc_pad + (src % npc)).astype(np.int64)
    core = dst // npc
    loc = dst % npc
    tl = loc // P
    off = loc % P

    key = core * tpc + tl
    counts = np.bincount(key, minlength=NCORES * tpc).reshape(NCORES, tpc)
    ch = np.maximum(1, -(-counts.max(axis=0) // P)).astype(np.int64)
    coloff = np.zeros(tpc + 1, np.int64)
    np.cumsum(ch, out=coloff[1:])
    ncols = int(coloff[-1])

    esrc = np.zeros((NCORES, ncols * P), np.int32)
    edst = np.full((NCORES, ncols * P), -1.0, np.float32)

    order = np.argsort(key, kind="stable")
    sk = key[order]
    first = np.r_[True, sk[1:] != sk[:-1]]
    idx_of_first = np.where(first)[0]
    grp_id = np.cumsum(first) - 1
    rank = np.arange(len(sk)) - idx_of_first[grp_id]
    slot = coloff[tl[order]] * P + rank
    esrc[core[order], slot] = srcpad[order].astype(np.int32)
    edst[core[order], slot] = off[order].astype(np.float32)

    esrc = np.ascontiguousarray(esrc.reshape(NCORES, ncols, P).transpose(0, 2, 1))
    edst = np.ascontiguousarray(edst.reshape(NCORES, ncols, P).transpose(0, 2, 1))

    # per-node 1/max(indegree,1), laid out [core][partition, tile]
    cnt = np.bincount(dst, minlength=n_nodes).astype(np.float32)
    recip = np.zeros((NCORES, npc_pad), np.float32)
    for c in range(NCORES):
        recip[c, :npc] = 1.0 / np.maximum(cnt[c * npc : (c + 1) * npc], 1.0)
    recip = np.ascontiguousarray(recip.reshape(NCORES, tpc, P).transpose(0, 2, 1))
    return ch, coloff, ncols, esrc, edst, recip


def _gather_groups(ch, coloff, tpc, gmax):
    """Group consecutive dst tiles so each group's gather is one indirect
    DMA of at most gmax columns (gmax*128 rows)."""
    groups = []
    t = 0
    while t < tpc:
        t0 = t
        cols = 0
        while t < tpc and cols + ch[t] <= gmax:
            cols += ch[t]
            t += 1
        groups.append((t0, t, int(coloff[t0]), int(coloff[t])))
    return groups


def _build_program(tpc, ncols, ch, coloff, groups, n_all_pad):
    from concourse import bacc, bass, mybir, tile

    npc_pad = tpc * P
    f32 = mybir.dt.float32
    i32 = mybir.dt.int32

    nc = bacc.Bacc(
        "TRN2", target_bir_lowering=False, debug=False, num_devices=NCORES
    )

    xg = nc.declare_dram_parameter("xg", [n_all_pad, P], f32, isOutput=False)
    xown = nc.declare_dram_parameter("xown", [npc_pad, P], f32, isOutput=False)
    esrc_d = nc.declare_dram_parameter("esrc", [P, ncols], i32, isOutput=False)
    edst_d = nc.declare_dram_parameter("edst", [P, ncols], f32, isOutput=False)
    wl1_d = nc.declare_dram_parameter("wl1", [P, P], f32, isOutput=False)
    wr1_d = nc.declare_dram_parameter("wr1", [P, P], f32, isOutput=False)
    wl2_d = nc.declare_dram_parameter("wl2", [P, P], f32, isOutput=False)
    wr2_d = nc.declare_dram_parameter("wr2", [P, P], f32, isOutput=False)
    bias1_d = nc.declare_dram_parameter("bias1", [P, P], f32, isOutput=False)
    bias2_d = nc.declare_dram_parameter("bias2", [P, P], f32, isOutput=False)
    iota_d = nc.declare_dram_parameter("iota", [P, P], f32, isOutput=False)
    ident_d = nc.declare_dram_parameter("ident", [P, P], f32, isOutput=False)
    recip_d = nc.declare_dram_parameter("recip", [P, tpc], f32, isOutput=False)
    out_d = nc.declare_dram_parameter("out", [npc_pad, P], f32, isOutput=True)

    gmax = max(g[3] - g[2] for g in groups)

    with tile.TileContext(nc) as tc:
        with (
            tc.tile_pool(name="const", bufs=1) as cpool,
            tc.tile_pool(name="meta", bufs=1) as mpool,
            tc.tile_pool(name="gath", bufs=2) as gpool,
            tc.tile_pool(name="work", bufs=3) as wpool,
            tc.tile_pool(name="psacc", bufs=2, space="PSUM") as ps_acc,
            tc.tile_pool(name="psself", bufs=2, space="PSUM") as ps_self,
            tc.tile_pool(name="psh", bufs=2, space="PSUM") as ps_h,
            tc.tile_pool(name="dram", bufs=1, space="DRAM") as dpool,
        ):
            def load_const(dram_ap, shape, dtype=f32, name=None):
                t = cpool.tile(shape, dtype, name=name)
                nc.sync.dma_start(out=t[:], in_=dram_ap)
                return t

            wl1 = load_const(wl1_d[:], [P, P], name="wl1")
            wr1 = load_const(wr1_d[:], [P, P], name="wr1")
            wl2 = load_const(wl2_d[:], [P, P], name="wl2")
            wr2 = load_const(wr2_d[:], [P, P], name="wr2")
            bias1 = load_const(bias1_d[:], [P, P], name="bias1")
            bias2 = load_const(bias2_d[:], [P, P], name="bias2")
            iota = load_const(iota_d[:], [P, P], name="iota")
            ident = load_const(ident_d[:], [P, P], name="ident")
            recip = load_const(recip_d[:], [P, tpc], name="recip")
            esrc = mpool.tile([P, ncols], i32, name="esrc")
            nc.sync.dma_start(out=esrc[:], in_=esrc_d[:])
            edst = mpool.tile([P, ncols], f32, name="edst")
            nc.sync.dma_start(out=edst[:], in_=edst_d[:])

            h_bounce = dpool.tile([npc_pad, P], f32, name="h_bounce")
            h_full = dpool.tile(
                [n_all_pad, P], f32, name="h_full", addr_space="Shared"
            )

            def layer(src_table, self_src, dst_dram, wl, wr, bias, relu):
                for (t0, t1, c0, c1) in groups:
                    g_sb = gpool.tile([P, gmax * P], f32, tag="gath")
                    # HW indirect DMA consumes ONE offset per partition, so
                    # gather 128 rows per instruction (one per chunk column).
                    for cc in range(c0, c1):
                        nc.gpsimd.indirect_dma_start(
                            out=g_sb[:, (cc - c0) * P : (cc - c0 + 1) * P],
                            out_offset=None,
                            in_=src_table[:],
                            in_offset=bass.IndirectOffsetOnAxis(
                                ap=esrc[:, cc : cc + 1], axis=0
                            ),
                        )
                    for t in range(t0, t1):
                        cht = int(ch[t])
                        tc0 = int(coloff[t]) - c0
                        ind = wpool.tile([P, cht, P], f32, tag="ind")
                        nc.vector.tensor_tensor(
                            out=ind[:],
                            in0=edst[:, coloff[t] : coloff[t] + cht, None]
                            .to_broadcast([P, cht, P]),
                            in1=iota[:, None, :].to_broadcast([P, cht, P]),
                            op=mybir.AluOpType.is_equal,
                        )
                        acc = ps_acc.tile([P, P], f32, tag="acc")
                        for k in range(cht):
                            nc.tensor.matmul(
                                out=acc[:],
                                lhsT=g_sb[:, (tc0 + k) * P : (tc0 + k + 1) * P],
                                rhs=ind[:, k, :],
                                start=(k == 0),
                                stop=(k == cht - 1),
                            )
                        # self term: x_own[t] transposed via PE
                        xo = wpool.tile([P, P], f32, tag="xo")
                        nc.sync.dma_start(
                            out=xo[:], in_=self_src[t * P : (t + 1) * P, :]
                        )
                        selfT_ps = ps_self.tile([P, P], f32, tag="selfT")
                        nc.tensor.transpose(
                            out=selfT_ps[:], in_=xo[:], identity=ident[:]
                        )
                        selfT = wpool.tile([P, P], f32, tag="selfT_sb")
                        nc.vector.tensor_copy(out=selfT[:], in_=selfT_ps[:])
                        aggT = wpool.tile([P, P], f32, tag="aggT_sb")
                        nc.vector.tensor_copy(out=aggT[:], in_=acc[:])
                        h1 = ps_h.tile([P, P], f32, tag="h1")
                        nc.tensor.matmul(
                            out=h1[:], lhsT=aggT[:], rhs=wl[:],
                            start=True, stop=True,
                        )
                        h2 = ps_h.tile([P, P], f32, tag="h2")
                        nc.tensor.matmul(
                            out=h2[:], lhsT=selfT[:], rhs=wr[:],
                            start=True, stop=True,
                        )
                        hsb = wpool.tile([P, P], f32, tag="hsb")
                        nc.vector.tensor_scalar_mul(
                            out=hsb[:], in0=h1[:], scalar1=recip[:, t : t + 1]
                        )
                        nc.vector.tensor_add(out=hsb[:], in0=hsb[:], in1=h2[:])
                        nc.vector.tensor_add(out=hsb[:], in0=hsb[:], in1=bias[:])
                        if relu:
                            nc.scalar.activation(
                                out=hsb[:], in_=hsb[:],
                                func=mybir.ActivationFunctionType.Relu,
                            )
                        nc.sync.dma_start(
                            out=dst_dram[t * P : (t + 1) * P, :], in_=hsb[:]
                        )

            layer(xg, xown, h_bounce, wl1, wr1, bias1, relu=True)
            nc.gpsimd.collective_compute(
                "AllGather",
                mybir.AluOpType.bypass,
                replica_groups=[list(range(NCORES))],
                ins=[h_bounce[:]],
                outs=[h_full[:]],
            )
            layer(h_full, h_bounce, out_d, wl2, wr2, bias2, relu=False)

    return nc


def run(x, edge_index, W_l1, b_l1, W_r1, W_l2, b_l2, W_r2, trace=False):
    n_nodes = x.shape[0]
    assert n_nodes % NCORES == 0
    npc = n_nodes // NCORES
    tpc = -(-npc // P)
    npc_pad = tpc * P
    n_all_pad = NCORES * npc_pad
    gmax = int(os.environ.get("SAGE_GMAX", "24"))

    ch, coloff, ncols, esrc, edst, recip = _prep_edges(
        edge_index, n_nodes, npc, tpc
    )
    groups = _gather_groups(ch, coloff, tpc, gmax)

    x = np.asarray(x, np.float32)
    x_pad = np.zeros((n_all_pad, P), np.float32)
    for c in range(NCORES):
        x_pad[c * npc_pad : c * npc_pad + npc] = x[c * npc : (c + 1) * npc]

    common = {
        "xg": x_pad,
        "wl1": np.asarray(W_l1, np.float32),
        "wr1": np.asarray(W_r1, np.float32),
        "wl2": np.asarray(W_l2, np.float32),
        "wr2": np.asarray(W_r2, np.float32),
        "bias1": np.ascontiguousarray(
            np.broadcast_to(np.asarray(b_l1, np.float32), (P, P))
        ),
        "bias2": np.ascontiguousarray(
            np.broadcast_to(np.asarray(b_l2, np.float32), (P, P))
        ),
        "iota": np.ascontiguousarray(
            np.broadcast_to(np.arange(P, dtype=np.float32), (P, P))
        ),
        "ident": np.eye(P, dtype=np.float32),
    }
    in_maps = []
    for c in range(NCORES):
        m = dict(common)
        m["xown"] = np.ascontiguousarray(x_pad[c * npc_pad : (c + 1) * npc_pad])
        m["esrc"] = esrc[c]
        m["edst"] = edst[c]
        m["recip"] = recip[c]
        in_maps.append(m)

    nc = _build_program(tpc, ncols, ch, coloff, groups, n_all_pad)
    nc.finalize()

    from concourse.bass_utils import run_bass_kernel_spmd

    res = run_bass_kernel_spmd(
        nc, in_maps, list(range(NCORES)), trace=trace,
    )
    out = np.empty((n_nodes, P), np.float32)
    for c in range(NCORES):
        out[c * npc : (c + 1) * npc] = res.results[c]["out"][:npc]
    return out, res


def kernel(x, edge_index, W_l1, b_l1, W_r1, W_l2, b_l2, W_r2):
    out, _ = run(x, edge_index, W_l1, b_l1, W_r1, W_l2, b_l2, W_r2)
    return out

